# revision 23
# baseline (speedup 1.0000x reference)
"""Batched dense attention (B=16, L=2048, D=128) on 8 TRN2 NeuronCores.

Data-parallel over the batch dim: each core gets 2 batches and computes a
full softmax(QK^T/sqrt(D)) plus context = A @ V locally; no collectives.

Outputs:
  attention_weights (16, 2048, 2048) float32
  context           (16, 2048, 128)  float32
"""

import math
from contextlib import ExitStack

import numpy as np

B, LQ, LKV, D = 16, 2048, 2048, 128
NCORES = 8
BPC = B // NCORES          # batches per core
QT = LQ // 128             # q-tiles per batch
KC = LKV // 128            # kv chunks of 128
SCALE = 1.0 / math.sqrt(D)

# A-output precision: "bf16" halves HBM write traffic (values still good to
# ~2^-9 relative); "f32" is exact.
A_DTYPE = "bf16"
# QK matmul dtype: "f32r" is full-rate fp32 (reduced-precision multiply),
# "f32" is exact but 4x slower on the PE.
QK_DTYPE = "f32r"
# Of the 16 P^T chunks per q-tile, how many go through the PE-transpose path
# (the rest use the DMA xbar transpose). Balances PE vs DMA load.
PE_TRANSPOSE_CHUNKS = 16
# AV runs this many q-tile iterations behind its exp/transpose.
AV_SHIFT = 3

_CACHE: dict = {}


def _build(repeat: int = 1):
    import concourse.bass as bass
    import concourse.tile as tile
    from concourse import bacc, mybir
    from concourse.masks import make_identity

    f32 = mybir.dt.float32
    bf16 = mybir.dt.bfloat16
    a_dt = bf16 if A_DTYPE == "bf16" else f32
    qk_dt = mybir.dt.float32r if QK_DTYPE == "f32r" else f32

    nc = bacc.Bacc("TRN2", debug=False, num_devices=NCORES)
    q = nc.dram_tensor("q", [BPC, LQ, D], f32, kind="ExternalInput")
    k = nc.dram_tensor("k", [BPC, LKV, D], f32, kind="ExternalInput")
    v = nc.dram_tensor("v", [BPC, LKV, D], f32, kind="ExternalInput")
    aw = nc.dram_tensor("aw", [BPC, LQ, LKV], a_dt, kind="ExternalOutput")
    cx = nc.dram_tensor("cx", [BPC, LQ, D], f32, kind="ExternalOutput")

    with tile.TileContext(nc) as tc, ExitStack() as ctx:
        const = ctx.enter_context(tc.tile_pool(name="const", bufs=1))
        setup = ctx.enter_context(tc.tile_pool(name="setup", bufs=2))
        loop = ctx.enter_context(tc.tile_pool(name="loop", bufs=2))
        stats = ctx.enter_context(tc.tile_pool(name="stats", bufs=4))
        psum = ctx.enter_context(tc.tile_pool(name="psum", bufs=1, space="PSUM"))
        psum2 = ctx.enter_context(tc.tile_pool(name="psum2", bufs=2, space="PSUM"))

        ident_f = const.tile([128, 128], f32, tag="idf")
        make_identity(nc, ident_f[:])
        ident_b = const.tile([128, 128], bf16, tag="idb")
        make_identity(nc, ident_b[:])

        for _ in range(repeat):
            # software pipeline: AV of tile (b,t) runs AV_SHIFT iterations
            # late; PE transposes of tile t run one iteration late.
            av_pend = []   # (b, t, pt_sb, vb, recip)
            tr_pend = None  # (p_sb, pt_sb)
            npe = PE_TRANSPOSE_CHUNKS

            def do_av(pend):
                pb, pt, pt_sb, pvb, precip, cx_all = pend
                cx_ps = psum2.tile([128, 128], f32, tag="cx")
                for c in range(KC):
                    nc.tensor.matmul(
                        cx_ps[:],
                        pt_sb[:, c, :],
                        pvb[:, c * 128:(c + 1) * 128],
                        start=(c == 0), stop=(c == KC - 1),
                    )
                nc.vector.tensor_scalar_mul(cx_all[:, pt, :], cx_ps[:], precip[:])
                if pt == QT - 1:
                    nc.sync.dma_start(
                        cx.ap()[pb].rearrange("(t p) d -> p t d", p=128), cx_all[:]
                    )

            def do_petrans(pend):
                p_prev, pt_prev = pend
                if npe == 0:
                    return
                pt_ps = psum.tile([128, npe * 128], bf16, tag="ptps")
                for c in range(npe):
                    nc.tensor.transpose(
                        pt_ps[:, c * 128:(c + 1) * 128],
                        p_prev[:, c * 128:(c + 1) * 128],
                        ident_b[:],
                    )
                nc.vector.tensor_copy(
                    pt_prev[:, 0:npe, :].rearrange("p c d -> p (c d)"), pt_ps[:]
                )

            def load_batch(bi):
                ld_qn = setup.tile([128, QT, 128], f32, tag="qn")
                nc.sync.dma_start(ld_qn[:], q.ap()[bi].rearrange("(c p) d -> p c d", p=128))
                ld_kn = setup.tile([128, KC, 128], f32, tag="kn")
                nc.sync.dma_start(ld_kn[:], k.ap()[bi].rearrange("(c p) d -> p c d", p=128))
                ld_vn = setup.tile([128, KC, 128], f32, tag="vn")
                nc.sync.dma_start(ld_vn[:], v.ap()[bi].rearrange("(c p) d -> p c d", p=128))
                return ld_qn, ld_kn, ld_vn

            loaded = load_batch(0)
            for b in range(BPC):
                # ---- per-batch setup: build QT/KT, V (bf16) from prefetched
                qn, kn, vn = loaded

                vb = setup.tile([128, KC * 128], bf16, tag="vb")
                nc.vector.tensor_copy(vb[:], vn[:].rearrange("p c d -> p (c d)"))
                cx_all = setup.tile([128, QT, 128], f32, tag="cxall")

                qt_sb = setup.tile([128, QT * 128], qk_dt, tag="qt")
                kt_sb = setup.tile([128, KC * 128], qk_dt, tag="kt")
                for dst, src in ((qt_sb, qn), (kt_sb, kn)):
                    for h in range(2):
                        tp = psum.tile([128, 1024], f32, tag=f"s{h}")
                        for c in range(8):
                            nc.tensor.transpose(
                                tp[:, c * 128:(c + 1) * 128], src[:, h * 8 + c, :], ident_f[:]
                            )
                        nc.scalar.copy(dst[:, h * 1024:(h + 1) * 1024], tp[:])

                # ---- q-tile loop
                for t in range(QT):
                    lhs = qt_sb[:, t * 128:(t + 1) * 128]
                    s_half = []
                    for h in range(2):
                        s_ps = psum.tile([128, 1024], f32, tag=f"s{h}")
                        for c2 in range(2):
                            nc.tensor.matmul(
                                s_ps[:, c2 * 512:(c2 + 1) * 512],
                                lhs,
                                kt_sb[:, (h * 2 + c2) * 512:(h * 2 + c2 + 1) * 512],
                                start=True, stop=True,
                            )
                        s_half.append(s_ps)

                    if len(av_pend) >= AV_SHIFT:
                        do_av(av_pend.pop(0))
                    if tr_pend is not None:
                        do_petrans(tr_pend)
                        tr_pend = None
                    if t == 8 and b + 1 < BPC:
                        loaded = load_batch(b + 1)

                    p_sb = loop.tile([128, LKV], bf16, tag="p", bufs=8)
                    hsums = []
                    for h in range(2):
                        hs = stats.tile([128, 1], f32, tag=f"hsum{h}")
                        nc.scalar.activation(
                            p_sb[:, h * 1024:(h + 1) * 1024], s_half[h][:],
                            mybir.ActivationFunctionType.Exp,
                            scale=SCALE, accum_out=hs[:],
                        )
                        hsums.append(hs)

                    pt_sb = loop.tile([128, KC, 128], bf16, tag="ptsb", bufs=8)
                    if npe < KC:
                        nc.sync.dma_start(
                            pt_sb[:, npe:, :],
                            p_sb[:, npe * 128:],
                            transpose=True,
                        )

                    sums = stats.tile([128, 1], f32, tag="sums")
                    nc.vector.tensor_add(sums[:], hsums[0][:], hsums[1][:])
                    recip = stats.tile([128, 1], f32, tag="recip")
                    nc.vector.reciprocal(recip[:], sums[:])

                    a_sb = loop.tile([128, LKV], a_dt, tag="a", bufs=4)
                    nc.vector.tensor_scalar_mul(a_sb[:], p_sb[:], recip[:])
                    nc.sync.dma_start(aw.ap()[b, t * 128:(t + 1) * 128, :], a_sb[:])

                    tr_pend = (p_sb, pt_sb)
                    av_pend.append((b, t, pt_sb, vb, recip, cx_all))

            if tr_pend is not None:
                do_petrans(tr_pend)
                tr_pend = None
            while av_pend:
                do_av(av_pend.pop(0))

    nc.compile()
    return nc


def _get_nc(repeat: int = 1):
    key = (repeat, A_DTYPE, QK_DTYPE)
    if key not in _CACHE:
        _CACHE[key] = _build(repeat)
    return _CACHE[key]


def kernel(query: np.ndarray, key: np.ndarray, value: np.ndarray):
    from concourse.bass_utils import run_bass_kernel_spmd

    nc = _get_nc()
    q = np.ascontiguousarray(query, dtype=np.float32)
    k = np.ascontiguousarray(key, dtype=np.float32)
    v = np.ascontiguousarray(value, dtype=np.float32)
    in_maps = [
        {"q": q[c * BPC:(c + 1) * BPC], "k": k[c * BPC:(c + 1) * BPC],
         "v": v[c * BPC:(c + 1) * BPC]}
        for c in range(NCORES)
    ]
    res = run_bass_kernel_spmd(nc, in_maps, core_ids=list(range(NCORES)))
    weights = np.concatenate([res.results[c]["aw"] for c in range(NCORES)], axis=0)
    if weights.dtype != np.float32:
        weights = weights.astype(np.float32)
    context = np.concatenate([res.results[c]["cx"] for c in range(NCORES)], axis=0)
    return weights, context
